# revision 44
# baseline (speedup 1.0000x reference)
"""DDGCRN cell on 8 TRN2 NeuronCores — data-parallel over batch.

v5: instances advance in PAIRS (partition bases 0/32) so that
  - the A-build's K=10 matmuls run row-packed via tile_position: the two
    pair members' V live at array row strips 0 and 32 and their chunk
    matmuls execute concurrently (~2x PE throughput on the A-build);
  - the hypernet runs as col/row-tiled matmul pairs with ONE activation
    per layer over the pair (Act op cost is free-dim-bound, so pairing
    halves it); one fused STT produces both vpre's; one tanh both V's;
  - Mb (emb*t*d*s*o) is built for a batch-pair in 4 TTs instead of 8;
  - the d-chain runs on [128,16] pair tiles (one op for both instances);
    one transpose/flatten serves both dB broadcasts;
  - xrep / bp / wx matmuls are row- or col-packed pairs.
Plus v2-v4 structure: all wide elementwise on DVE (GPSIMD shares a DVE
SBUF port — keep it to DMAs); STT/TS fusions for xp, the cand-natural
build and the filt bias; packed input DMAs; dependency-ordered DMA
queues; dB by stride-0 broadcast DMA; P6/zout + z-transposes at the
step head so the PE never waits on the yd->subs DVE chain.

Math per instance:
  filt = hypernet MLP (transposed-feature layout, bf16)
  V = tanh(emb*time*day*speed*occupy*filt)      (10, 883)
  A = relu(V V^T) (883,883 symmetric) + fused row-sums (ACT accum_out)
  d = rsqrt(rowsum) via fast-inverse-sqrt + 1 Newton step (DVE only)
  y^T = (d*xs)^T A  (A symmetric); yd = y^T * dB; Lx = x0 - yd
  out^T = bp.T@embT + WX.T@zx40 + sum_e WZS_e.T @ (embSS_e * S2)  (PSUM acc)

All matmuls bf16 (PSUM f32); inputs pre-cast/pre-transposed on host (pure
layout/dtype prep). Output written transposed bf16, un-transposed on host.
"""

import sys, os

sys.path.insert(0, "/opt/trn_rl_repo")

import numpy as np
import ml_dtypes
from contextlib import ExitStack

import concourse.bass as bass
import concourse.bacc as bacc
import concourse.mybir as mybir
from concourse import tile
from concourse.alu_op_type import AluOpType
from concourse.bass_types import AP
from concourse.bass_utils import run_bass_kernel_spmd

AF = mybir.ActivationFunctionType
F32 = mybir.dt.float32
BF16 = mybir.dt.bfloat16
I32 = mybir.dt.int32
BF16_NP = ml_dtypes.bfloat16

B, N, DIN, DOUT, E, CHEB = 64, 883, 2, 64, 10, 2
C = DIN + DOUT  # 66
NCORES = 8
BL = B // NCORES  # 8 batches per core
NT = (N + 127) // 128  # 7 row tiles
OG, OU = 2 * DOUT, DOUT  # 128, 64
SPLITS = [(0, 512), (512, N - 512)]
RSQRT_MAGIC = 0x5F3759DF
MUL = AluOpType.mult
ADD = AluOpType.add
XW = 68            # packed natural-xs tile pitch (64 state + 2 x + pad)
NP = 896           # padded column count
PB = 32            # pair member b's partition base

# pair schedule: update-pair >= 3 supersteps after its gate-pair (same-step
# works too because glue_rest is emitted before P1 of the next pair)
PSEQ = [("g", 0), ("g", 1), ("g", 2), ("u", 0), ("g", 3), ("u", 1),
        ("u", 2), ("u", 3)]
PAIRS = [(0, 1), (2, 3), (4, 5), (6, 7)]


def _pt(nt):
    return min(128, N - nt * 128)


def _bcast(ap, reps):
    """Repeat a single-partition AP `reps` times via a stride-0 dim."""
    return AP(ap.tensor, ap.offset, [ap.ap[0], [0, reps]] + list(ap.ap[1:]))


def _build_body(tc, ctx, nc, P):
    def pool(name, bufs, space="SBUF"):
        return ctx.enter_context(tc.tile_pool(name=name, bufs=bufs, space=space))

    wp = pool("wp", 1)        # static weights
    dat = pool("dat", 2)      # per-batch-pair DMA loads
    act = pool("act", 2)      # per-instance/pair intermediates
    arp = pool("arp", 15)     # relu(A) tiles: 14 + 1 in flight
    xnp = pool("xnp", 8)      # packed natural xs tiles (1/batch)
    znp = pool("znp", 28)     # natural z tiles (14/gate-pair, 2 pairs live)
    xpp = pool("xpp", 15)     # d*xg natural tiles
    zp = pool("zp", 20)       # packed Z tiles (128, N), 20/pair
    dnp = pool("dnp", 2)      # rowsum/d pair helpers
    psp = pool("psp", 1, space="PSUM")  # op/yT/xrp (2-bank tiles)
    psa = pool("psa", 3, space="PSUM")  # A / hypernet / misc (2-bank tiles)

    # ---------------- static setup ----------------
    ident_f = wp.tile([128, 128], F32, tag="identf", name="ident_f")
    nc.sync.dma_start(ident_f[:, :], P["ident"][:, :])
    identB64 = wp.tile([128, 64], BF16, tag="identb64", name="identB64")
    nc.sync.dma_start(identB64[:, :], P["identB64"][:, :])

    def load_bf(pname, shape, tag, q=nc.sync):
        t = wp.tile(list(shape), BF16, tag=tag, name=pname + "_t")
        q.dma_start(t[:, :], P[pname][:, :])
        return t

    embT2 = load_bf("embT2", (42, N), "embT2")
    embX40_2 = load_bf("embX40_2", (104, N), "embX40_2")
    sel4_2 = load_bf("sel4_2", (36, 40), "sel4_2")
    # heavy statics on the gpsimd DMA queue: the first batches' input loads
    # on the sync queue must not queue behind ~20 descriptors
    embSS = []
    for e in range(E):
        t = wp.tile([128, N], BF16, tag=f"embSS{e}", name=f"embSS{e}")
        nc.gpsimd.dma_start(t[:, :], P["embSS"][e * 128:(e + 1) * 128, :])
        embSS.append(t)
    wzs = {}
    for br, On in (("g", OG), ("u", OU)):
        tiles = []
        for e in range(E):
            t = wp.tile([128, On], BF16, tag=f"wzs{br}{e}", name=f"wzs{br}{e}")
            nc.gpsimd.dma_start(t[:, :], P[f"wzs_{br}"][e * 128:(e + 1) * 128, :])
            tiles.append(t)
        wzs[br] = tiles
    wx2 = {"g": load_bf("wx2_g", (104, OG), "wx2g", nc.gpsimd),
           "u": load_bf("wx2_u", (104, OU), "wx2u", nc.gpsimd)}
    bp2 = {"g": load_bf("bp2_g", (42, OG), "bp2g"),
           "u": load_bf("bp2_u", (42, OU), "bp2u")}
    fc = {}
    for br in ("g", "u"):
        fc[("w1", br)] = load_bf(f"fc1w_{br}", (C, 16), f"fc1w{br}")
        fc[("w2", br)] = load_bf(f"fc2w2_{br}", (48, 2), f"fc2w{br}")
        fc[("w3", br)] = load_bf(f"fc3w2_{br}", (34, E), f"fc3w{br}")
        for nm, shape in (("b1", (48, 1)), ("b2", (34, 1)), ("b3", (42, 1))):
            t = wp.tile(list(shape), F32, tag=f"fc{nm}{br}", name=f"fc{nm}{br}")
            nc.sync.dma_start(t[:, :], P[f"fc{nm}2_{br}"][:, :])
            fc[(nm, br)] = t

    # ---------------- per-pair state ----------------
    ST = {}   # (br, pair_idx) -> dict
    BAT = {}  # pair_idx -> dict of per-batch-pair tiles

    def batch_load(pi):
        """DMA the batch-pair's inputs; build the pair Mb in 4 DVE TTs."""
        ba, bb = PAIRS[pi]
        d = {}
        for j, b in enumerate((ba, bb)):
            xsn = xnp.tile([128, NT * XW], BF16, tag="xsn", name=f"xsn{b}")
            nc.sync.dma_start(xsn[:, :], P["xsn"][b, :, :])
            d[f"xsn{j}"] = xsn
            csT = act.tile([C, N], BF16, tag="csT", name=f"csT{b}", bufs=8)
            nc.sync.dma_start(csT[:, :], P["csT"][b, :, :])
            d[f"csT{j}"] = csT
            S2g = act.tile([128, N], BF16, tag="S2g", name=f"S2g{b}", bufs=6)
            nc.sync.dma_start(S2g[64:128, :], P["stateT"][b, :, :])
            d[f"S2g{j}"] = S2g
        td4 = dat.tile([42, 4 * NP], BF16, tag="td4", name=f"td4{pi}", bufs=1)
        nc.sync.dma_start(td4[0:E, :], P["tdso4"][ba, :, :])
        nc.sync.dma_start(td4[PB:PB + E, :], P["tdso4"][bb, :, :])
        d["td4"] = td4
        BAT[pi] = d

    def mb_build(pi):
        """Mb TT chain, deferred to just before its vpre consumer so it
        never delays the critical yd->subs->Z chain at the step head."""
        td4 = BAT[pi].pop("td4")
        p1 = act.tile([42, N], BF16, tag="p1", name=f"p1_{pi}", bufs=1)
        nc.vector.tensor_mul(p1[:, :], td4[:, 0:N], td4[:, NP:NP + N])
        p2 = act.tile([42, N], BF16, tag="p2", name=f"p2_{pi}", bufs=1)
        nc.vector.tensor_mul(p2[:, :], td4[:, 2 * NP:2 * NP + N],
                             td4[:, 3 * NP:3 * NP + N])
        p3 = act.tile([42, N], BF16, tag="p3", name=f"p3_{pi}", bufs=1)
        nc.vector.tensor_mul(p3[:, :], p1[:, :], p2[:, :])
        Mb = act.tile([42, N], BF16, tag="Mb", name=f"Mb{pi}", bufs=3)
        nc.vector.tensor_mul(Mb[:, :], p3[:, :], embT2[:, :])
        BAT[pi]["Mb"] = Mb

    def P1_load(inst):
        br, pi = inst
        if br == "g":
            batch_load(pi)
            st = ST[inst] = {}
            st["x0T"] = [BAT[pi]["csT0"], BAT[pi]["csT1"]]
        else:
            st = ST[inst]  # created by glue(gate): has x0T=[CU_a, CU_b]
        rs = dnp.tile([128, 16], F32, tag="rs0", name=f"rs0{br}{pi}")
        nc.vector.memset(rs[:, :], 1.0)
        st["rs"] = rs

    def P1_l1(inst):
        br, pi = inst
        st = ST[inst]
        h1p = psa.tile([48, 896], F32, tag="psB", name=f"h1p{br}{pi}")
        h1 = act.tile([48, N], BF16, tag="h1", name=f"h1{br}{pi}", bufs=1)
        for j, base in ((0, 0), (1, PB)):
            xg2 = st["x0T"][j]
            for s0, sl in SPLITS:
                nc.tensor.matmul(h1p[base:base + 16, s0:s0 + sl],
                                 fc[("w1", br)][:, :], xg2[:, s0:s0 + sl],
                                 start=True, stop=True,
                                 tile_position=(0, base))
        nc.scalar.activation(h1[:, :], h1p[0:48, 0:N],
                             AF.Sigmoid, bias=fc[("b1", br)][:, :])
        st["h1"] = h1

    def P1_l2(inst):
        br, pi = inst
        st = ST[inst]
        h1 = st["h1"]
        h2p = psa.tile([34, 896], F32, tag="psB", name=f"h2p{br}{pi}")
        h2 = act.tile([34, N], BF16, tag="h2", name=f"h2{br}{pi}", bufs=1)
        for base in (0, PB):
            for s0, sl in SPLITS:
                nc.tensor.matmul(h2p[base:base + 2, s0:s0 + sl],
                                 fc[("w2", br)][base:base + 16, :],
                                 h1[base:base + 16, s0:s0 + sl],
                                 start=True, stop=True,
                                 tile_position=(base, base))
        nc.scalar.activation(h2[:, :], h2p[0:34, 0:N],
                             AF.Sigmoid, bias=fc[("b2", br)][:, :])
        st["h2"] = h2

    def P1_l3V(inst):
        br, pi = inst
        st = ST[inst]
        h2 = st["h2"]
        h3p = psa.tile([42, 896], F32, tag="psB", name=f"h3p{br}{pi}")
        for base in (0, PB):
            for s0, sl in SPLITS:
                nc.tensor.matmul(h3p[base:base + E, s0:s0 + sl],
                                 fc[("w3", br)][base:base + 2, :],
                                 h2[base:base + 2, s0:s0 + sl],
                                 start=True, stop=True,
                                 tile_position=(base, base))
        if br == "g":
            mb_build(pi)
        # both vpre's in ONE fused STT; both V's in ONE tanh
        vpre = act.tile([42, N], BF16, tag="vpre", name=f"vpre{br}{pi}", bufs=1)
        nc.vector.scalar_tensor_tensor(vpre[:, :], h3p[0:42, 0:N],
                                       fc[("b3", br)][:, :],
                                       BAT[pi]["Mb"][:, :], ADD, MUL)
        V4 = act.tile([42, N], BF16, tag="V4", name=f"V4{br}{pi}")
        nc.scalar.activation(V4[:, :], vpre[:, :], AF.Tanh)
        st["V4"] = V4

    def P2_chunk(inst, kts):
        """Row-packed pair A-build: both members' chunk matmuls execute
        concurrently in array row strips 0 and 32."""
        br, pi = inst
        st = ST[inst]
        V4, rsh = st["V4"], st["rs"]
        ara = st.setdefault("ar0", [])
        arb = st.setdefault("ar1", [])
        for kt in kts:
            p = _pt(kt)
            apsA = psa.tile([128, 896], F32, tag="psB", name=f"apA{br}{pi}{kt}")
            apsB = psa.tile([128, 896], F32, tag="psB", name=f"apB{br}{pi}{kt}")
            arta = arp.tile([128, N], BF16, tag="ar", name=f"arA{br}{pi}{kt}")
            artb = arp.tile([128, N], BF16, tag="ar", name=f"arB{br}{pi}{kt}")
            for s0, sl in SPLITS:
                nc.tensor.matmul(apsA[:p, s0:s0 + sl],
                                 V4[0:E, kt * 128:kt * 128 + p],
                                 V4[0:E, s0:s0 + sl], start=True, stop=True,
                                 tile_position=(0, 0))
                nc.tensor.matmul(apsB[:p, s0:s0 + sl],
                                 V4[PB:PB + E, kt * 128:kt * 128 + p],
                                 V4[PB:PB + E, s0:s0 + sl], start=True,
                                 stop=True, tile_position=(PB, 0))
            nc.scalar.activation(arta[:p, 0:N], apsA[:p, 0:N],
                                 AF.Relu, accum_out=rsh[:p, kt:kt + 1])
            nc.scalar.activation(artb[:p, 0:N], apsB[:p, 0:N],
                                 AF.Relu, accum_out=rsh[:p, 8 + kt:9 + kt])
            ara.append(arta)
            arb.append(artb)

    def P3(inst):
        """Pair d-chain on [128,16] tiles; one transpose/flatten for both
        dB broadcasts; per-instance xp via per-partition TS/STT."""
        br, pi = inst
        st = ST[inst]
        rsall = st["rs"]
        tsh = dnp.tile([128, 16], F32, tag="tsh", name=f"tsh{br}{pi}")
        nc.vector.tensor_scalar(tsh[:, :].bitcast(I32), rsall[:, :].bitcast(I32),
                                1, None, AluOpType.logical_shift_right)
        tnot = dnp.tile([128, 16], F32, tag="tnot", name=f"tnot{br}{pi}")
        nc.vector.tensor_scalar(tnot[:, :].bitcast(I32), tsh[:, :].bitcast(I32),
                                -1, None, AluOpType.bitwise_xor)
        d0 = dnp.tile([128, 16], F32, tag="d0", name=f"d0{br}{pi}")
        nc.vector.tensor_scalar(d0[:, :].bitcast(I32), tnot[:, :].bitcast(I32),
                                RSQRT_MAGIC + 1, None, AluOpType.add)
        sq = dnp.tile([128, 16], F32, tag="sq", name=f"sq{br}{pi}")
        nc.vector.tensor_mul(sq[:, :], d0[:, :], d0[:, :])
        hx = dnp.tile([128, 16], F32, tag="hx", name=f"hx{br}{pi}")
        nc.vector.tensor_mul(hx[:, :], sq[:, :], rsall[:, :])
        cf = dnp.tile([128, 16], F32, tag="cf", name=f"cf{br}{pi}")
        nc.vector.tensor_scalar(cf[:, :], hx[:, :], -0.5, 1.5,
                                AluOpType.mult, AluOpType.add)
        dcat = dnp.tile([128, 16], F32, tag="dcat", name=f"dcat{br}{pi}")
        nc.vector.tensor_mul(dcat[:, :], d0[:, :], cf[:, :])
        st["dcat"] = dcat
        tp = psa.tile([128, 128], F32, tag="psB", name=f"dtp{br}{pi}")
        nc.tensor.transpose(tp[:16, :128], dcat[:, :], ident_f[:, :])
        drs = act.tile([16, 128], BF16, tag="drs", name=f"drs{br}{pi}", bufs=1)
        nc.scalar.copy(drs[:, :], tp[:16, :128])
        dBs = []
        for j in (0, 1):
            drow = act.tile([1, NP], BF16, tag="drow", name=f"drow{br}{pi}{j}",
                            bufs=2)
            nc.gpsimd.dma_start(drow[0:1, :], drs[8 * j:8 * j + 7, :])
            dB = act.tile([C, NP], BF16, tag="dB", name=f"dB{br}{pi}{j}", bufs=3)
            nc.gpsimd.dma_start(dB[:, :], _bcast(drow[0:1, 0:NP], C))
            dBs.append(dB)
        st["dB"] = dBs
        # x4 pair tile: rows {2:4, 34:36} = x^T via DMA; {0:2, 32:34} = Lx-x
        x4 = act.tile([36, 896], BF16, tag="x4", name=f"x4{br}{pi}", bufs=2)
        ba, bb = PAIRS[pi]
        nc.gpsimd.dma_start(x4[2:4, 0:N], P["xT"][ba, :, :])
        nc.gpsimd.dma_start(x4[PB + 2:PB + 4, 0:N], P["xT"][bb, :, :])
        st["x4"] = x4
        xp = [[], []]
        for j in (0, 1):
            xsn = BAT[pi][f"xsn{j}"]
            dcol = 8 * j
            for kt in range(NT):
                p = _pt(kt)
                c0 = kt * XW
                xpt = xpp.tile([128, C], BF16, tag="xp",
                               name=f"xp{br}{pi}{j}{kt}")
                if br == "g":
                    nc.vector.tensor_scalar(xpt[:p, :], xsn[:p, c0:c0 + C],
                                            dcat[:p, dcol + kt:dcol + kt + 1],
                                            None, MUL)
                else:
                    zn = ST[("g", pi)]["zn"][j]
                    nc.vector.scalar_tensor_tensor(
                        xpt[:p, 0:C], zn[kt][:p, 0:C],
                        dcat[:p, dcol + kt:dcol + kt + 1],
                        xsn[:p, c0:c0 + C], MUL, MUL)
                xp[j].append(xpt)
        st["xp"] = xp

    def P4P5(inst, j):
        """One pair member's yT matmuls + the whole dependent DVE chain
        (yd -> x4/S2 subs -> Z tiles): member a's chain runs on the DVE
        while the PE streams member b's yT."""
        br, pi = inst
        st = ST[inst]
        base = PB * j
        yt = psp.tile([C, 896], F32, tag="psA", name=f"yt{br}{pi}{j}")
        ar, xpj = st[f"ar{j}"], st["xp"][j]
        for kt in range(NT):
            p = _pt(kt)
            for s0, sl in SPLITS:
                nc.tensor.matmul(yt[:C, s0:s0 + sl], xpj[kt][:p, :],
                                 ar[kt][:p, s0:s0 + sl],
                                 start=(kt == 0), stop=(kt == NT - 1))
        yd = act.tile([C, N], BF16, tag="yd", name=f"yd{br}{pi}{j}", bufs=2)
        nc.vector.tensor_mul(yd[:, :], yt[:C, 0:N], st["dB"][j][:, 0:N])
        x4 = st["x4"]
        x0T = st["x0T"][j]
        nc.vector.tensor_sub(x4[base:base + 2, 0:N],
                             x0T[64:66, :], yd[64:66, :])
        S2 = BAT[pi][f"S2g{j}"] if br == "g" else st["S2u"][j]
        nc.vector.tensor_sub(S2[0:64, :], x0T[0:64, :], yd[0:64, :])
        zt = st.setdefault("zt", [[], []])
        for e in range(E):
            z = zp.tile([128, N], BF16, tag="Z", name=f"Z{br}{pi}{j}{e}")
            nc.vector.tensor_mul(z[:, :], embSS[e][:, :], S2[:, :])
            zt[j].append(z)

    def P5c(inst):
        """Row/col-packed xrep matmul pair + both zx TTs."""
        br, pi = inst
        st = ST[inst]
        x4 = st["x4"]
        zx = act.tile([104, 896], BF16, tag="zx", name=f"zx{br}{pi}", bufs=2)
        xrp = psp.tile([104, 896], F32, tag="psA", name=f"xrp{br}{pi}")
        for s0, sl in SPLITS:
            nc.tensor.matmul(xrp[0:40, s0:s0 + sl], sel4_2[0:4, :],
                             x4[0:4, s0:s0 + sl], start=True, stop=True,
                             tile_position=(0, 0))
            nc.tensor.matmul(xrp[64:104, s0:s0 + sl], sel4_2[PB:PB + 4, :],
                             x4[PB:PB + 4, s0:s0 + sl], start=True, stop=True,
                             tile_position=(PB, 64))
        nc.vector.tensor_mul(zx[0:40, 0:N], embX40_2[0:40, :], xrp[0:40, 0:N])
        nc.vector.tensor_mul(zx[64:104, 0:N], embX40_2[64:104, :],
                             xrp[64:104, 0:N])
        st["zx"] = zx

    def P6(inst):
        """Final per-node einsum per member + output activations.

        Update pairs COLUMN-pack: O=64, so member a lands in array col
        strips 0-1 and member b in 2-3 of ONE [128,896] psum — the two
        members' K=128 wzs matmuls run concurrently on disjoint cells
        with a shared stationary, halving the update einsum's PE slots."""
        br, pi = inst
        st = ST[inst]
        zx, zt = st["zx"], st["zt"]
        if br == "g":
            ops = []
            for j, base, xb in ((0, 0, 0), (1, PB, 64)):
                op = psp.tile([OG, 896], F32, tag="psA", name=f"op{br}{pi}{j}")
                for s0, sl in SPLITS:
                    nc.tensor.matmul(op[:OG, s0:s0 + sl],
                                     bp2[br][base:base + E, :],
                                     embT2[base:base + E, s0:s0 + sl],
                                     start=True, stop=False,
                                     tile_position=(base, 0))
                    for e in range(E):
                        nc.tensor.matmul(op[:OG, s0:s0 + sl], wzs[br][e][:, :],
                                         zt[j][e][:, s0:s0 + sl], start=False,
                                         stop=False)
                    nc.tensor.matmul(op[:OG, s0:s0 + sl],
                                     wx2[br][xb:xb + 40, :],
                                     zx[xb:xb + 40, s0:s0 + sl],
                                     start=False, stop=True,
                                     tile_position=(xb, 0))
                ops.append(op)
            psrc = [(ops[0], 0), (ops[1], 0)]
        else:
            op = psp.tile([128, 896], F32, tag="psA", name=f"op{br}{pi}")
            for s0, sl in SPLITS:
                for j, ob in ((0, 0), (1, 64)):
                    base, xb = (0, 0) if j == 0 else (PB, 64)
                    nc.tensor.matmul(op[ob:ob + OU, s0:s0 + sl],
                                     bp2[br][base:base + E, :],
                                     embT2[base:base + E, s0:s0 + sl],
                                     start=True, stop=False,
                                     tile_position=(base, ob))
                for e in range(E):
                    for j, ob in ((0, 0), (1, 64)):
                        nc.tensor.matmul(op[ob:ob + OU, s0:s0 + sl],
                                         wzs[br][e][:, :],
                                         zt[j][e][:, s0:s0 + sl], start=False,
                                         stop=False, tile_position=(0, ob))
                for j, ob in ((0, 0), (1, 64)):
                    xb = 0 if j == 0 else 64
                    nc.tensor.matmul(op[ob:ob + OU, s0:s0 + sl],
                                     wx2[br][xb:xb + 40, :],
                                     zx[xb:xb + 40, s0:s0 + sl],
                                     start=False, stop=True,
                                     tile_position=(xb, ob))
            psrc = [(op, 0), (op, 64)]
        On = OG if br == "g" else OU
        outf = AF.Sigmoid if br == "g" else AF.Tanh
        zouts = []
        for j in (0, 1):
            # padded to 896 cols so downstream full-block reads are legal
            pt, ob = psrc[j]
            zout = act.tile([On, NP], BF16, tag=f"zout{br}",
                            name=f"zout{br}{pi}{j}", bufs=(6 if br == "g" else 2))
            nc.scalar.activation(zout[:, :], pt[ob:ob + On, 0:NP], outf)
            zouts.append(zout)
        st["zout"] = zouts

    def glue_pe(inst):
        """Natural z tiles for the update pair's fused xp."""
        br, pi = inst
        if br != "g":
            return
        zn_all = [[], []]
        for j in (0, 1):
            zr = ST[inst]["zout"][j]
            for nt in range(NT):
                p = _pt(nt)
                zps = psa.tile([128, 64], BF16, tag="psB", name=f"znp{pi}{j}{nt}")
                nc.tensor.transpose(zps[:p, :DOUT],
                                    zr[64:128, nt * 128:nt * 128 + p],
                                    identB64[64:128, :])
                # 66 cols: 64 transposed z + 2 ones, so the update xp is
                # ONE fused STT over all 66 features (x cols ride as 1*d*x)
                zn = znp.tile([128, C], BF16, tag="zn", name=f"zn{pi}{j}{nt}")
                nc.vector.tensor_copy(zn[:p, 0:DOUT], zps[:p, :DOUT])
                nc.vector.memset(zn[:p, DOUT:C], 1.0)
                zn_all[j].append(zn)
        ST[inst]["zn"] = zn_all

    def glue_rest(inst):
        br, pi = inst
        if br == "g":
            ust = ST[("u", pi)] = {}
            S2us, CUs = [], []
            for j in (0, 1):
                zr = ST[inst]["zout"][j]
                S2g = BAT[pi][f"S2g{j}"]
                csT = BAT[pi][f"csT{j}"]
                S2u = act.tile([128, N], BF16, tag="S2u",
                               name=f"S2u{pi}{j}", bufs=4)
                nc.vector.tensor_mul(S2u[64:128, :], zr[64:128, 0:N],
                                     S2g[64:128, :])
                CU = act.tile([C, N], BF16, tag="CU", name=f"CU{pi}{j}", bufs=4)
                nc.vector.tensor_copy(CU[0:64, :], S2u[64:128, :])
                nc.vector.tensor_copy(CU[64:66, :], csT[64:66, :])
                S2us.append(S2u)
                CUs.append(CU)
            ust["S2u"] = S2us
            ust["x0T"] = CUs
            ust["zn"] = ST[inst]["zn"]
        else:
            for j in (0, 1):
                hc = ST[inst]["zout"][j]
                r = ST[("g", pi)]["zout"][j]
                csT = BAT[pi][f"csT{j}"]
                b = PAIRS[pi][j]
                t1 = act.tile([OU, N], BF16, tag="t1", name=f"t1_{b}", bufs=2)
                nc.vector.tensor_sub(t1[:, :], csT[0:64, :], hc[:, 0:N])
                t2 = act.tile([OU, N], BF16, tag="t2", name=f"t2_{b}", bufs=1)
                nc.vector.tensor_mul(t2[:, :], r[0:64, 0:N], t1[:, :])
                # reuse t1's storage for the result (dead after t2)
                nc.vector.tensor_add(t1[:, :], t2[:, :], hc[:, 0:N])
                nc.gpsimd.dma_start(P["out"][b, :, :], t1[:, :])

    # ---------------- pipeline driver ----------------
    M = len(PSEQ)
    P1_load(PSEQ[0]); P1_l1(PSEQ[0]); P1_l2(PSEQ[0]); P1_l3V(PSEQ[0])
    for s in range(M + 1):
        nxt = PSEQ[s + 1] if s + 1 < M else None
        cur = PSEQ[s] if s < M else None
        if 0 <= s - 2 < M - 1:
            P6(PSEQ[s - 2])
            glue_pe(PSEQ[s - 2])
            if PSEQ[s - 2][0] == "g":
                glue_rest(PSEQ[s - 2])
        if nxt:
            P1_load(nxt)
            P1_l1(nxt)
        if 0 <= s - 1 < M:
            P4P5(PSEQ[s - 1], 0)
        if cur:
            P2_chunk(cur, range(0, 3))
        if nxt:
            P1_l2(nxt)
        if 0 <= s - 1 < M:
            P4P5(PSEQ[s - 1], 1)
        if cur:
            P2_chunk(cur, range(3, 7))
        if 0 <= s - 1 < M:
            P5c(PSEQ[s - 1])
        if 0 <= s - 2 < M - 1 and PSEQ[s - 2][0] == "u":
            # epilogues have slack (feed only the out DMA): run after the
            # critical yd->subs->Z chains, before the drain pull-in
            glue_rest(PSEQ[s - 2])
        if s - 1 == M - 1:
            # last pair: drain immediately instead of waiting a superstep
            P6(PSEQ[M - 1])
            glue_rest(PSEQ[M - 1])
        if nxt:
            P1_l3V(nxt)
        if cur:
            P3(cur)


def build_nc():
    nc = bacc.Bacc()
    P = {}

    def dp(name, shape, dtype=F32, out=False):
        P[name] = nc.declare_dram_parameter(name, list(shape), dtype, isOutput=out)

    dp("xsn", (BL, 128, NT * XW), BF16)
    dp("csT", (BL, C, N), BF16)
    dp("stateT", (BL, DOUT, N), BF16)
    dp("xT", (BL, DIN, N), BF16)
    dp("tdso4", (BL, E, 4 * NP), BF16)
    dp("embT2", (42, N), BF16)
    dp("embSS", (E * 128, N), BF16)
    dp("embX40_2", (104, N), BF16)
    dp("sel4_2", (36, 40), BF16)
    dp("wzs_g", (E * 128, OG), BF16)
    dp("wzs_u", (E * 128, OU), BF16)
    dp("wx2_g", (104, OG), BF16)
    dp("wx2_u", (104, OU), BF16)
    dp("bp2_g", (42, OG), BF16)
    dp("bp2_u", (42, OU), BF16)
    for br in ("g", "u"):
        dp(f"fc1w_{br}", (C, 16), BF16)
        dp(f"fc2w2_{br}", (48, 2), BF16)
        dp(f"fc3w2_{br}", (34, E), BF16)
        dp(f"fcb12_{br}", (48, 1))
        dp(f"fcb22_{br}", (34, 1))
        dp(f"fcb32_{br}", (42, 1))
    dp("ident", (128, 128))
    dp("identB64", (128, 64), BF16)
    dp("out", (BL, OU, N), BF16, out=True)
    with tile.TileContext(nc) as tc:
        with ExitStack() as ctx:
            _build_body(tc, ctx, nc, P)
    nc.finalize()
    return nc


_NC_CACHE = {}


def _get_nc():
    if "nc" not in _NC_CACHE:
        _NC_CACHE["nc"] = build_nc()
    return _NC_CACHE["nc"]


def _pair2(a, rows, base=PB):
    """Stack `a` (rows0) twice at partition bases 0 and `base`."""
    out = np.zeros((base + a.shape[0],) + a.shape[1:], a.dtype)
    out[0:a.shape[0]] = a
    out[base:base + a.shape[0]] = a
    return out


def _make_in_maps(inputs):
    f32 = lambda a: np.ascontiguousarray(a, dtype=np.float32)
    bf = lambda a: np.ascontiguousarray(np.asarray(a, dtype=np.float32).astype(BF16_NP))
    x = f32(inputs["x"])
    state = f32(inputs["state"])
    emb = f32(inputs["node_embeddings"])
    time, day = f32(inputs["time"]), f32(inputs["day"])
    speed, occupy = f32(inputs["speed"]), f32(inputs["occupy"])

    embT = emb.T                                      # (E, N)
    embSS = np.repeat(embT[:, None, :], 128, axis=1).reshape(E * 128, N)
    embX40 = np.repeat(embT[:, None, :], 4, axis=1).reshape(E * 4, N)
    sel4 = np.tile(np.eye(4, dtype=np.float32), (1, E))  # (4, 40)
    perm_feat = list(range(DIN, C)) + [0, 1]          # state-first

    def pack_w(wpool, operm):
        wp = wpool[..., operm]
        wzs = np.concatenate([wp[:, 1, DIN:, :], wp[:, 0, DIN:, :]], axis=1)
        wzs = wzs.reshape(E * 128, -1)
        wxp = np.stack([wp[:, 1, 0, :], wp[:, 1, 1, :],
                        wp[:, 0, 0, :], wp[:, 0, 1, :]], axis=1)
        wxp = wxp.reshape(E * 4, -1)
        return wzs, wxp

    operm_g = list(range(DOUT, OG)) + list(range(DOUT))  # [r; z]
    wzs_g, wx_g = pack_w(inputs["gate_wpool"], operm_g)
    wzs_u, wx_u = pack_w(inputs["update_wpool"], list(range(OU)))

    identB64 = np.zeros((128, 64), np.float32)
    identB64[64:128, :] = np.eye(64, dtype=np.float32)

    shared = {
        "embT2": bf(_pair2(embT, E)),
        "embSS": bf(embSS),
        "embX40_2": bf(_pair2(embX40, 40, 64)),
        "sel4_2": bf(_pair2(sel4, 4)),
        "wzs_g": bf(wzs_g),
        "wzs_u": bf(wzs_u),
        "wx2_g": bf(_pair2(wx_g, 40, 64)),
        "wx2_u": bf(_pair2(wx_u, 40, 64)),
        "bp2_g": bf(_pair2(inputs["gate_bpool"][:, operm_g], E)),
        "bp2_u": bf(_pair2(inputs["update_bpool"], E)),
        "ident": np.eye(128, dtype=np.float32),
        "identB64": bf(identB64),
    }
    for br, pre in (("g", "gate"), ("u", "update")):
        shared[f"fc1w_{br}"] = bf(inputs[f"{pre}_fc1_w"][perm_feat, :])
        shared[f"fc2w2_{br}"] = bf(_pair2(inputs[f"{pre}_fc2_w"], 16))
        shared[f"fc3w2_{br}"] = bf(_pair2(inputs[f"{pre}_fc3_w"], 2))
        shared[f"fcb12_{br}"] = f32(_pair2(inputs[f"{pre}_fc1_b"].reshape(16, 1), 16))
        shared[f"fcb22_{br}"] = f32(_pair2(inputs[f"{pre}_fc2_b"].reshape(2, 1), 2))
        shared[f"fcb32_{br}"] = f32(_pair2(inputs[f"{pre}_fc3_b"].reshape(E, 1), E))

    in_maps = []
    for c in range(NCORES):
        sl = slice(c * BL, (c + 1) * BL)
        m = dict(shared)
        xs, ss = x[sl], state[sl]
        xsn = np.zeros((BL, 128, NT * XW), np.float32)
        for nt in range(NT):
            p = _pt(nt)
            xsn[:, :p, nt * XW:nt * XW + DOUT] = ss[:, nt * 128:nt * 128 + p, :]
            xsn[:, :p, nt * XW + DOUT:nt * XW + C] = xs[:, nt * 128:nt * 128 + p, :]
        m["xsn"] = bf(xsn)
        m["csT"] = bf(np.concatenate([ss.transpose(0, 2, 1),
                                      xs.transpose(0, 2, 1)], axis=1))
        m["stateT"] = bf(ss.transpose(0, 2, 1))
        m["xT"] = bf(xs.transpose(0, 2, 1))
        td4 = np.zeros((BL, E, 4 * NP), np.float32)
        for j, a in enumerate((time, day, speed, occupy)):
            td4[:, :, j * NP:j * NP + N] = a[sl].transpose(0, 2, 1)
        m["tdso4"] = bf(td4)
        in_maps.append(m)
    return in_maps


def _run(inputs, trace=False):
    nc = _get_nc()
    in_maps = _make_in_maps(inputs)
    res = run_bass_kernel_spmd(nc, in_maps, core_ids=list(range(NCORES)), trace=trace)
    out = np.concatenate(
        [np.asarray(res.results[i]["out"]).astype(np.float32).transpose(0, 2, 1)
         for i in range(NCORES)],
        axis=0,
    )
    return np.ascontiguousarray(out), res


def kernel(**inputs):
    out, _ = _run(inputs, trace=False)
    return out
